# revision 2
# baseline (speedup 1.0000x reference)
"""v3: all-bf16 Bengio03ResNetBiLm kernel.

Changes vs v2 (f32r baseline, 1049us):
- every matmul operand is bf16 (same 1 cyc/row as f32r, but half the
  LDWEIGHTS time, half the SBUF footprint, and 2-byte copies);
- transposes stay on the PE but in bf16 (1 cyc/row vs 1.5 for f32r;
  DMA XBAR transposes cost 1.2us of sync-engine ucode each — measured —
  so they are NOT used);
- x0 ingest: one gpsimd cast-DMA (f32->bf16) per batch, then PE
  transposes;
- B2's residual-add stt writes straight into the next layer's x buffer
  (the GpSimd bounce copies are gone);
- LN stat matmuls are emitted after B1(prev) so the relu/sq ACT->DVE
  chain of the current group is never on the PE's critical path;
- sq moves from ACT to DVE (bf16 in/out);
- store path: stt emits bf16, PE transpose (bf16) to psum, ACT/DVE
  cast-copy to f32 staging, then DMA to HBM.
"""

import contextlib

import numpy as np
import ml_dtypes

import concourse.bacc as bacc
import concourse.tile as tile
from concourse import mybir

F32 = mybir.dt.float32
BF16 = mybir.dt.bfloat16
AF = mybir.ActivationFunctionType
ALU = mybir.AluOpType

W = 3
H = 256
HC = 2
EPS = 1e-6
NPBF = ml_dtypes.bfloat16


def prep_weights(inputs, L):
    f32 = np.float32
    LB = 2 * L
    wpT = np.zeros((L, 2, 4, HC, 128, HC, 128), f32)  # [l, br, j, c, p, m, n]
    ctxb_col = np.zeros((LB, 128, HC), f32)
    w1pT = np.zeros((L, 2, HC, 128, HC, 128), f32)    # [l, br, c, p, m, n]
    b1p = np.zeros((LB, 128, HC), f32)
    cs1_s = np.zeros((128, LB, HC, 128), f32)          # rows 32g = colsum(w1')
    w2T = np.zeros((L, 2, HC, 128, HC, 128), f32)
    b2col = np.zeros((LB, 128, HC), f32)
    padT = np.zeros((L, HC, 128, 2 * W), f32)

    for l in range(L):
        for br, (Wc, bc, g, beta, w1, b1, w2_, b2) in enumerate(
            (
                (inputs["fwd_W"][l], inputs["fwd_b"][l], inputs["ln_f_g"][l],
                 inputs["ln_f_b"][l], inputs["ffn_f_w1"][l], inputs["ffn_f_b1"][l],
                 inputs["ffn_f_w2"][l], inputs["ffn_f_b2"][l]),
                (inputs["bwd_W"][l], inputs["bwd_b"][l], inputs["ln_b_g"][l],
                 inputs["ln_b_b"][l], inputs["ffn_b_w1"][l], inputs["ffn_b_b1"][l],
                 inputs["ffn_b_w2"][l], inputs["ffn_b_b2"][l]),
            )
        ):
            lb = l * 2 + br
            wpT[l, br] = np.asarray(Wc, f32).reshape(4, HC, 128, HC, 128)
            ctxb_col[lb] = np.asarray(bc, f32).reshape(HC, 128).T
            w1f = np.asarray(g, f32)[:, None] * np.asarray(w1, f32)
            b1f = np.asarray(b1, f32) + np.asarray(beta, f32) @ np.asarray(w1, f32)
            w1pT[l, br] = w1f.reshape(HC, 128, HC, 128)
            b1p[lb] = b1f.reshape(HC, 128).T
            cs1 = w1f.sum(0).reshape(HC, 128)  # colsum
            for gg in range(4):
                cs1_s[32 * gg, lb] = cs1
            w2T[l, br] = np.asarray(w2_, f32).reshape(HC, 128, HC, 128)
            b2col[lb] = np.asarray(b2, f32).reshape(HC, 128).T
        fp = np.asarray(inputs["fwd_pad"][l], f32)
        bp = np.asarray(inputs["bwd_pad"][l], f32)
        padT[l] = np.concatenate([fp, bp], 0).T.reshape(HC, 128, 2 * W)

    ones4 = np.zeros((128, 128), f32)
    for gg in range(4):
        ones4[32 * gg] = 1.0
    ohcols = np.zeros((4, 128, 128), f32)
    for gg in range(4):
        ohcols[gg, :, 32 * gg] = 1.0
    bf = NPBF
    return dict(
        wpT=wpT.astype(bf), ctxb_col=ctxb_col,
        w1pT=w1pT.astype(bf), b1p=b1p, cs1_s=cs1_s.astype(bf),
        w2T=w2T.astype(bf), b2col=b2col, padT=padT.astype(bf),
        ones4=ones4.astype(bf), ohcols=ohcols.astype(bf))


def build_nc(B_local, S_, L, arsqrt=True):
    NG = S_ // 512
    SP = S_ + 2 * W
    LB = 2 * L

    nc = bacc.Bacc()
    dr = {}
    dr["x0"] = nc.dram_tensor("x0", [B_local, S_, H], F32, kind="ExternalInput")
    dr["wpT"] = nc.dram_tensor("wpT", [L, 2, 4, HC, 128, HC, 128], BF16,
                               kind="ExternalInput")
    dr["ctxb_col"] = nc.dram_tensor("ctxb_col", [LB, 128, HC], F32,
                                    kind="ExternalInput")
    dr["w1pT"] = nc.dram_tensor("w1pT", [L, 2, HC, 128, HC, 128], BF16,
                                kind="ExternalInput")
    dr["b1p"] = nc.dram_tensor("b1p", [LB, 128, HC], F32, kind="ExternalInput")
    dr["cs1_s"] = nc.dram_tensor("cs1_s", [128, LB, HC, 128], BF16,
                                 kind="ExternalInput")
    dr["w2T"] = nc.dram_tensor("w2T", [L, 2, HC, 128, HC, 128], BF16,
                               kind="ExternalInput")
    dr["b2col"] = nc.dram_tensor("b2col", [LB, 128, HC], F32, kind="ExternalInput")
    dr["padT"] = nc.dram_tensor("padT", [L, HC, 128, 2 * W], BF16,
                                kind="ExternalInput")
    dr["ones4"] = nc.dram_tensor("ones4", [128, 128], BF16, kind="ExternalInput")
    dr["ohcols"] = nc.dram_tensor("ohcols", [4, 128, 128], BF16,
                                  kind="ExternalInput")
    dr["out"] = nc.dram_tensor("out", [L, B_local, S_, 2 * H], F32,
                               kind="ExternalOutput")

    with tile.TileContext(nc) as tc:
        _body(nc, tc, B_local, S_, L, NG, SP, LB, dr, arsqrt)
    nc.compile()
    return nc


def _body(nc, tc, B_local, S_, L, NG, SP, LB, dr, arsqrt):
    ctx = contextlib.ExitStack()
    with ctx:
        consts = ctx.enter_context(tc.tile_pool(name="consts", bufs=1))
        wstream = ctx.enter_context(tc.tile_pool(name="wstream", bufs=2))
        xbufs = ctx.enter_context(tc.tile_pool(name="xbufs", bufs=1))
        x0tm_p = ctx.enter_context(tc.tile_pool(name="x0tm", bufs=2))
        h_p = ctx.enter_context(tc.tile_pool(name="h", bufs=2))
        sq_p = ctx.enter_context(tc.tile_pool(name="sq", bufs=2))
        rows_p = ctx.enter_context(tc.tile_pool(name="rows", bufs=2))
        f1_p = ctx.enter_context(tc.tile_pool(name="f1", bufs=2))
        tmp_p = ctx.enter_context(tc.tile_pool(name="tmp", bufs=2))
        xn_p = ctx.enter_context(tc.tile_pool(name="xn", bufs=2))
        tm_p = ctx.enter_context(tc.tile_pool(name="tm", bufs=3))
        pm = ctx.enter_context(tc.tile_pool(name="pm", bufs=5, space="PSUM"))
        ps_st = ctx.enter_context(tc.tile_pool(name="ps_st", bufs=1, space="PSUM"))
        ps_misc = ctx.enter_context(tc.tile_pool(name="ps_misc", bufs=1, space="PSUM"))

        # ---- constants ----
        from concourse.masks import make_identity
        identf = consts.tile([128, 128], F32)
        make_identity(nc, identf[:])
        ident = consts.tile([128, 128], BF16)
        nc.vector.tensor_copy(out=ident[:], in_=identf[:])
        eps_t = consts.tile([128, 1], F32)
        nc.vector.memset(eps_t[:], EPS)
        ones4 = consts.tile([128, 128], BF16)
        nc.gpsimd.dma_start(ones4[:], dr["ones4"].ap())
        ohcols = consts.tile([128, 4, 128], BF16)
        nc.gpsimd.dma_start(ohcols[:], dr["ohcols"].ap().rearrange("g p m -> p g m"))
        cs1_s = consts.tile([128, LB, HC, 128], BF16)
        nc.gpsimd.dma_start(cs1_s[:], dr["cs1_s"].ap())
        ctxb_col = consts.tile([128, LB, HC], F32)
        nc.sync.dma_start(ctxb_col[:], dr["ctxb_col"].ap().rearrange("a p m -> p a m"))
        b1p = consts.tile([128, LB, HC], F32)
        nc.sync.dma_start(b1p[:], dr["b1p"].ap().rearrange("a p m -> p a m"))
        b2col = consts.tile([128, LB, HC], F32)
        nc.sync.dma_start(b2col[:], dr["b2col"].ap().rearrange("a p m -> p a m"))
        padT_s = consts.tile([128, L, HC, 2 * W], BF16)
        nc.gpsimd.dma_start(padT_s[:], dr["padT"].ap().rearrange("l c p w -> p l c w"))
        w1pT_s = consts.tile([128, L, 2, HC, HC, 128], BF16)
        nc.gpsimd.dma_start(
            w1pT_s[:], dr["w1pT"].ap().rearrange("l b c p m n -> p l b c m n"))
        w2T_s = consts.tile([128, L, 2, HC, HC, 128], BF16)
        nc.gpsimd.dma_start(
            w2T_s[:], dr["w2T"].ap().rearrange("l b c p m n -> p l b c m n"))

        # ---- per-batch buffer state ----
        bufs = {}   # b -> dict(x0, xA, xB)
        wp_tiles = {}

        def in_buf(b, l, br):
            d = bufs[b]
            return d["x0"] if l == 0 else (d["xA"][br] if l % 2 == 1 else d["xB"][br])

        def out_buf(b, l, br):
            d = bufs[b]
            return d["xA"][br] if l % 2 == 0 else d["xB"][br]

        # ---- unit emission helpers; unit = (b, l, br) ----
        state = {}  # unit -> dict(h_sb, st_sum, st_sq, negm, rstd)

        def prologue(u):
            b, l, br = u
            if l == 0 and br == 0:
                d = {}
                d["x0"] = xbufs.tile([128, HC, SP], BF16, tag="xB0",
                                     name=f"x0_fm_{b}")
                d["xA"] = [xbufs.tile([128, HC, SP], BF16, tag=f"xA{i}",
                                      name=f"xA{i}_{b}") for i in range(2)]
                d["xB"] = [xbufs.tile([128, HC, SP], BF16, tag=f"xB{i}",
                                      name=f"xB{i}_{b}") for i in range(2)]
                bufs[b] = d
                x0_fm = d["x0"]
                # token-major bf16 staging (gpsimd DMA casts f32->bf16),
                # then PE transposes into the feature-major buffer.
                x0tm = x0tm_p.tile([128, S_ // 128, H], BF16, tag="x0tm",
                                   name=f"x0tm_{b}")
                nc.gpsimd.dma_start(
                    x0tm[:],
                    dr["x0"].ap()[b].rearrange("(g p) h -> p g h", p=128))
                for g2 in range(S_ // 128):
                    pst = ps_misc.tile([128, 2, 128], BF16, tag="misc",
                                       name="pst0")
                    for c in range(HC):
                        nc.tensor.transpose(
                            pst[:, c, :],
                            x0tm[:, g2, c * 128 : (c + 1) * 128], ident[:])
                    col = W + g2 * 128
                    if g2 % 2 == 0:
                        nc.scalar.copy(
                            out=x0_fm[:, :, col : col + 128], in_=pst[:])
                    else:
                        nc.vector.tensor_copy(
                            out=x0_fm[:, :, col : col + 128], in_=pst[:])
            if br == 0:
                wpT = wstream.tile([128, 2, 4, HC, HC, 128], BF16, tag="wpT",
                                   name=f"wpT_{b}_{l}")
                nc.gpsimd.dma_start(
                    wpT[:],
                    dr["wpT"].ap()[l].rearrange("b j c p m n -> p b j c m n"))
                wp_tiles[(b, l)] = wpT
            if not (l == 0 and br == 1):
                buf = in_buf(b, l, br)
                for c in range(HC):
                    nc.gpsimd.tensor_copy(buf[:, c, 0:W], padT_s[:, l, c, 0:W])
                    nc.gpsimd.tensor_copy(
                        buf[:, c, S_ + W : S_ + 2 * W], padT_s[:, l, c, W : 2 * W])

        def emit_A_mm(u, g):
            # ctx matmuls + relu(h) + sq; the LN stat matmuls are emitted
            # separately (emit_A_stats) after B1(prev) to keep them off the
            # PE critical path while ACT/DVE produce h and sq.
            b, l, br = u
            lb = l * 2 + br
            xin = in_buf(b, l, br)
            wpT = wp_tiles[(b, l)]
            off = 0 if br == 0 else W
            st = state.setdefault(u, {})
            if g == 0:
                st["h_sb"] = h_p.tile([128, HC, S_], BF16, tag="h",
                                      name=f"h_{b}_{l}_{br}")
                st["st_sum"] = ps_st.tile([128, 512], F32, tag="st_sum",
                                          name=f"stsum_{b}_{l}_{br}")
                st["st_sq"] = ps_st.tile([128, 512], F32, tag="st_sq",
                                         name=f"stsq_{b}_{l}_{br}")
            h_sb = st["h_sb"]
            t0 = g * 512
            sqs = []
            for m in range(HC):
                psc = pm.tile([128, 512], F32, tag="pm", name="psc")
                for j in range(W + 1):
                    for c in range(HC):
                        nc.tensor.matmul(
                            psc[:], wpT[:, br, j, c, m, :],
                            xin[:, c, t0 + off + j : t0 + off + j + 512],
                            start=(j == 0 and c == 0),
                            stop=(j == W and c == HC - 1))
                nc.scalar.activation(
                    h_sb[:, m, t0 : t0 + 512], psc[:], AF.Relu,
                    bias=ctxb_col[:, lb, m : m + 1])
                sq = sq_p.tile([128, 512], BF16, tag="sq", name="sq")
                nc.vector.tensor_mul(sq[:], h_sb[:, m, t0 : t0 + 512],
                                     h_sb[:, m, t0 : t0 + 512])
                sqs.append(sq)
            st[("sqs", g)] = sqs

        def emit_A_stats(u, g):
            st = state[u]
            h_sb = st["h_sb"]
            sqs = st.pop(("sqs", g))
            t0 = g * 512
            for m in range(HC):
                nc.tensor.matmul(
                    st["st_sum"][:], ohcols[:, g % 4, :], h_sb[:, m, t0 : t0 + 512],
                    start=(g == 0 and m == 0), stop=(g == NG - 1 and m == HC - 1),
                    skip_group_check=True)
                nc.tensor.matmul(
                    st["st_sq"][:], ohcols[:, g % 4, :], sqs[m][:],
                    start=(g == 0 and m == 0), stop=(g == NG - 1 and m == HC - 1),
                    skip_group_check=True)

        def emit_R(u):
            st = state[u]
            negm = rows_p.tile([128, 512], BF16, tag="negm", name="negm")
            nc.vector.tensor_scalar_mul(negm[:], st["st_sum"][:], -1.0 / H)
            m2 = rows_p.tile([128, 512], F32, tag="rtmp", name="m2")
            nc.vector.tensor_mul(m2[:], negm[:], negm[:])
            v = rows_p.tile([128, 512], F32, tag="rtmp", name="v")
            nc.vector.scalar_tensor_tensor(
                out=v[:], in0=st["st_sq"][:], scalar=1.0 / H, in1=m2[:],
                op0=ALU.mult, op1=ALU.subtract)
            rstd = rows_p.tile([128, 512], BF16, tag="rstd", name="rstd")
            if arsqrt:
                nc.scalar.activation(
                    rstd[:], v[:], AF.Abs_reciprocal_sqrt, bias=eps_t[:])
            else:
                sig = rows_p.tile([128, 512], F32, tag="rtmp", name="sig")
                nc.scalar.activation(sig[:], v[:], AF.Sqrt, bias=eps_t[:])
                with nc.allow_low_precision(reason="bf16 rstd for matmul rhs"):
                    nc.vector.reciprocal(rstd[:], sig[:])
            st["negm"] = negm
            st["rstd"] = rstd

        def emit_B1(u, g):
            b, l, br = u
            lb = l * 2 + br
            st = state[u]
            h_sb, negm, rstd = st["h_sb"], st["negm"], st["rstd"]
            t0 = g * 512
            gp = 32 * (g % 4)
            psz = [None, None]
            for m in range(HC):
                psz[m] = pm.tile([128, 512], F32, tag="pm", name="psz")
                for c in range(HC):
                    nc.tensor.matmul(
                        psz[m][:], w1pT_s[:, l, br, c, m, :],
                        h_sb[:, c, t0 : t0 + 512], start=(c == 0), stop=False)
                nc.tensor.matmul(
                    psz[m][:], cs1_s[gp : gp + 1, lb, m, :], negm[gp : gp + 1, :],
                    start=False, stop=True, tile_position=(gp, 0))
            rb = pm.tile([128, 512], F32, tag="pm", name="rb")
            nc.tensor.matmul(
                rb[:], ones4[gp : gp + 1, :], rstd[gp : gp + 1, :],
                start=True, stop=True, tile_position=(gp, 0))
            rb_sb = tmp_p.tile([128, 512], F32, tag="rb_sb", name="rb_sb")
            nc.scalar.copy(out=rb_sb[:], in_=rb[:])
            f1_sb = f1_p.tile([128, HC, 512], BF16, tag="f1", name="f1_sb")
            for m in range(HC):
                t_sb = tmp_p.tile([128, 512], F32, tag="t_sb", name="t_sb")
                nc.vector.tensor_mul(t_sb[:], psz[m][:], rb_sb[:])
                nc.scalar.activation(
                    f1_sb[:, m, :], t_sb[:], AF.Relu, bias=b1p[:, lb, m : m + 1])
            st[("f1", g)] = f1_sb

        def emit_B2(u, g):
            b, l, br = u
            lb = l * 2 + br
            st = state[u]
            h_sb = st["h_sb"]
            f1_sb = st.pop(("f1", g))
            t0 = g * 512
            if l < L - 1:
                xn_dst = out_buf(b, l, br)
                xn_off = W + t0
            else:
                xn_dst = xn_p.tile([128, HC, 512], BF16, tag="xn", name="xn_st")
                xn_off = 0
            for m in range(HC):
                pso = pm.tile([128, 512], F32, tag="pm", name="pso")
                for c in range(HC):
                    nc.tensor.matmul(
                        pso[:], w2T_s[:, l, br, c, m, :], f1_sb[:, c, :],
                        start=(c == 0), stop=(c == HC - 1))
                nc.vector.scalar_tensor_tensor(
                    out=xn_dst[:, m, xn_off : xn_off + 512], in0=pso[:],
                    scalar=b2col[:, lb, m : m + 1],
                    in1=h_sb[:, m, t0 : t0 + 512], op0=ALU.add, op1=ALU.add)
            for s in range(4):
                pst = ps_misc.tile([128, 2, 128], BF16, tag="misc", name="pst")
                for m in range(HC):
                    so = xn_off + s * 128
                    nc.tensor.transpose(
                        pst[:, m, :], xn_dst[:, m, so : so + 128], ident[:])
                tmf = tm_p.tile([128, 2, 128], F32, tag="tmf", name="tmf")
                if s % 2 == 0:
                    nc.scalar.copy(out=tmf[:], in_=pst[:])
                else:
                    nc.vector.tensor_copy(out=tmf[:], in_=pst[:])
                nc.sync.dma_start(
                    dr["out"].ap()[l, b, t0 + s * 128 : t0 + (s + 1) * 128,
                                   br * H : (br + 1) * H],
                    tmf[:])

        # ---- software-pipelined unit stream ----
        # Per 512-token group: Amm(u_i, g) | B1(u_{i-1}, g) | stats(u_i, g) |
        # B2(u_{i-1}, g-1): the PE runs ctx matmuls, then B1 (independent of
        # this group's relu/sq), then the stats (whose ACT/DVE inputs are now
        # ready), then ffn2 — no stall on the relu/sq chain.
        units = [(b, l, br) for b in range(B_local) for l in range(L)
                 for br in range(2)]
        prev = None
        for u in units:
            prologue(u)
            for g in range(NG):
                emit_A_mm(u, g)
                if prev is not None:
                    emit_B1(prev, g)
                emit_A_stats(u, g)
                if prev is not None and g > 0:
                    emit_B2(prev, g - 1)
            emit_R(u)
            if prev is not None:
                emit_B2(prev, NG - 1)
                state.pop(prev)
            prev = u
        for g in range(NG):
            emit_B1(prev, g)
            if g > 0:
                emit_B2(prev, g - 1)
        emit_B2(prev, NG - 1)


# ---- SPMD wrapper ----
from concourse.bass_utils import run_bass_kernel_spmd

B, S, L_ = 32, 2048, 3
N_CORES = 8
B_LOCAL = B // N_CORES
_NC_CACHE = {}


def _get_nc():
    key = (B_LOCAL, S)
    if key not in _NC_CACHE:
        _NC_CACHE[key] = build_nc(B_LOCAL, S, L_)
    return _NC_CACHE[key]


def run(inputs, **spmd_kwargs):
    prep = prep_weights(inputs, L_)
    x = np.ascontiguousarray(np.asarray(inputs["inputs"], np.float32))
    nc = _get_nc()
    in_maps = []
    for core in range(N_CORES):
        m = {"x0": x[core * B_LOCAL : (core + 1) * B_LOCAL]}
        m.update(prep)
        in_maps.append(m)
    res = run_bass_kernel_spmd(nc, in_maps, list(range(N_CORES)), **spmd_kwargs)
    out = np.concatenate([res.results[i]["out"] for i in range(N_CORES)], axis=1)
    return out, res


def kernel(**inputs):
    out, _ = run(inputs)
    return out


# revision 3
# speedup vs baseline: 1.0624x; 1.0624x over previous
"""v3: all-bf16 Bengio03ResNetBiLm kernel.

Changes vs v2 (f32r baseline, 1049us):
- every matmul operand is bf16 (same 1 cyc/row as f32r, but half the
  LDWEIGHTS time, half the SBUF footprint, and 2-byte copies);
- transposes stay on the PE but in bf16 (1 cyc/row vs 1.5 for f32r;
  DMA XBAR transposes cost 1.2us of sync-engine ucode each — measured —
  so they are NOT used);
- x0 ingest: one gpsimd cast-DMA (f32->bf16) per batch, then PE
  transposes;
- B2's residual-add stt writes straight into the next layer's x buffer
  (the GpSimd bounce copies are gone);
- LN stat matmuls are emitted after B1(prev) so the relu/sq ACT->DVE
  chain of the current group is never on the PE's critical path;
- sq moves from ACT to DVE (bf16 in/out);
- store path: stt emits bf16, PE transpose (bf16) to psum, ACT/DVE
  cast-copy to f32 staging, then DMA to HBM.
"""

import contextlib

import numpy as np
import ml_dtypes

import concourse.bacc as bacc
import concourse.tile as tile
from concourse import mybir

F32 = mybir.dt.float32
BF16 = mybir.dt.bfloat16
FP8 = mybir.dt.float8e4
AF = mybir.ActivationFunctionType
ALU = mybir.AluOpType

W = 3
H = 256
HC = 2
EPS = 1e-6
NPBF = ml_dtypes.bfloat16


def prep_weights(inputs, L):
    f32 = np.float32
    LB = 2 * L
    wpT = np.zeros((L, 2, 4, HC, 128, HC, 128), f32)  # [l, br, j, c, p, m, n]
    ctxb_col = np.zeros((LB, 128, HC), f32)
    w1pT = np.zeros((L, 2, HC, 128, HC, 128), f32)    # [l, br, c, p, m, n]
    b1p = np.zeros((LB, 128, HC), f32)
    cs1_s = np.zeros((128, LB, HC, 128), f32)          # rows 32g = colsum(w1')
    w2T = np.zeros((L, 2, HC, 128, HC, 128), f32)
    b2col = np.zeros((LB, 128, HC), f32)
    padT = np.zeros((L, HC, 128, 2 * W), f32)

    for l in range(L):
        for br, (Wc, bc, g, beta, w1, b1, w2_, b2) in enumerate(
            (
                (inputs["fwd_W"][l], inputs["fwd_b"][l], inputs["ln_f_g"][l],
                 inputs["ln_f_b"][l], inputs["ffn_f_w1"][l], inputs["ffn_f_b1"][l],
                 inputs["ffn_f_w2"][l], inputs["ffn_f_b2"][l]),
                (inputs["bwd_W"][l], inputs["bwd_b"][l], inputs["ln_b_g"][l],
                 inputs["ln_b_b"][l], inputs["ffn_b_w1"][l], inputs["ffn_b_b1"][l],
                 inputs["ffn_b_w2"][l], inputs["ffn_b_b2"][l]),
            )
        ):
            lb = l * 2 + br
            wpT[l, br] = np.asarray(Wc, f32).reshape(4, HC, 128, HC, 128)
            ctxb_col[lb] = np.asarray(bc, f32).reshape(HC, 128).T
            w1f = np.asarray(g, f32)[:, None] * np.asarray(w1, f32)
            b1f = np.asarray(b1, f32) + np.asarray(beta, f32) @ np.asarray(w1, f32)
            w1pT[l, br] = w1f.reshape(HC, 128, HC, 128)
            b1p[lb] = b1f.reshape(HC, 128).T
            cs1 = w1f.sum(0).reshape(HC, 128)  # colsum
            for gg in range(4):
                cs1_s[32 * gg, lb] = cs1
            w2T[l, br] = np.asarray(w2_, f32).reshape(HC, 128, HC, 128)
            b2col[lb] = np.asarray(b2, f32).reshape(HC, 128).T
        fp = np.asarray(inputs["fwd_pad"][l], f32)
        bp = np.asarray(inputs["bwd_pad"][l], f32)
        padT[l] = np.concatenate([fp, bp], 0).T.reshape(HC, 128, 2 * W)

    # per-group padded stationaries: plane g is zero except row 32g
    cs1_g = np.zeros((128, 4, LB, HC, 128), f32)
    for gg in range(4):
        cs1_g[32 * gg, gg] = cs1_s[0]
    ones_g = np.zeros((128, 4, 128), f32)
    for gg in range(4):
        ones_g[32 * gg, gg] = 1.0
    # fp8 DoubleRow stat stationary: both k-planes select column 32g
    oh8 = np.zeros((128, 4, 2, 128), f32)
    for gg in range(4):
        oh8[:, gg, :, 32 * gg] = 1.0
    bf = NPBF
    f8 = ml_dtypes.float8_e4m3fn
    return dict(
        wpT=wpT.astype(bf), ctxb_col=ctxb_col,
        w1pT=w1pT.astype(bf), b1p=b1p,
        w2T=w2T.astype(bf), b2col=b2col, padT=padT.astype(bf),
        cs1_g=cs1_g.astype(bf), ones_g=ones_g.astype(bf),
        oh8=oh8.astype(f8))


def build_nc(B_local, S_, L, arsqrt=True):
    NG = S_ // 512
    SP = S_ + 2 * W
    LB = 2 * L

    nc = bacc.Bacc()
    dr = {}
    dr["x0"] = nc.dram_tensor("x0", [B_local, S_, H], F32, kind="ExternalInput")
    dr["wpT"] = nc.dram_tensor("wpT", [L, 2, 4, HC, 128, HC, 128], BF16,
                               kind="ExternalInput")
    dr["ctxb_col"] = nc.dram_tensor("ctxb_col", [LB, 128, HC], F32,
                                    kind="ExternalInput")
    dr["w1pT"] = nc.dram_tensor("w1pT", [L, 2, HC, 128, HC, 128], BF16,
                                kind="ExternalInput")
    dr["b1p"] = nc.dram_tensor("b1p", [LB, 128, HC], F32, kind="ExternalInput")
    dr["cs1_g"] = nc.dram_tensor("cs1_g", [128, 4, LB, HC, 128], BF16,
                                 kind="ExternalInput")
    dr["ones_g"] = nc.dram_tensor("ones_g", [128, 4, 128], BF16,
                                  kind="ExternalInput")
    dr["oh8"] = nc.dram_tensor("oh8", [128, 4, 2, 128], FP8,
                               kind="ExternalInput")
    dr["w2T"] = nc.dram_tensor("w2T", [L, 2, HC, 128, HC, 128], BF16,
                               kind="ExternalInput")
    dr["b2col"] = nc.dram_tensor("b2col", [LB, 128, HC], F32, kind="ExternalInput")
    dr["padT"] = nc.dram_tensor("padT", [L, HC, 128, 2 * W], BF16,
                                kind="ExternalInput")
    dr["out"] = nc.dram_tensor("out", [L, B_local, S_, 2 * H], F32,
                               kind="ExternalOutput")

    with tile.TileContext(nc) as tc:
        _body(nc, tc, B_local, S_, L, NG, SP, LB, dr, arsqrt)
    nc.compile()
    return nc


def _body(nc, tc, B_local, S_, L, NG, SP, LB, dr, arsqrt):
    ctx = contextlib.ExitStack()
    with ctx:
        consts = ctx.enter_context(tc.tile_pool(name="consts", bufs=1))
        wstream = ctx.enter_context(tc.tile_pool(name="wstream", bufs=2))
        xbufs = ctx.enter_context(tc.tile_pool(name="xbufs", bufs=1))
        x0tm_p = ctx.enter_context(tc.tile_pool(name="x0tm", bufs=2))
        h_p = ctx.enter_context(tc.tile_pool(name="h", bufs=2))
        sq_p = ctx.enter_context(tc.tile_pool(name="sq", bufs=2))
        rows_p = ctx.enter_context(tc.tile_pool(name="rows", bufs=2))
        f1_p = ctx.enter_context(tc.tile_pool(name="f1", bufs=2))
        tmp_p = ctx.enter_context(tc.tile_pool(name="tmp", bufs=2))
        xn_p = ctx.enter_context(tc.tile_pool(name="xn", bufs=2))
        tm_p = ctx.enter_context(tc.tile_pool(name="tm", bufs=3))
        pm = ctx.enter_context(tc.tile_pool(name="pm", bufs=5, space="PSUM"))
        ps_st = ctx.enter_context(tc.tile_pool(name="ps_st", bufs=1, space="PSUM"))
        ps_misc = ctx.enter_context(tc.tile_pool(name="ps_misc", bufs=1, space="PSUM"))

        # ---- constants ----
        from concourse.masks import make_identity
        identf = consts.tile([128, 128], F32)
        make_identity(nc, identf[:])
        ident = consts.tile([128, 128], BF16)
        nc.vector.tensor_copy(out=ident[:], in_=identf[:])
        eps_t = consts.tile([128, 1], F32)
        nc.vector.memset(eps_t[:], EPS)
        cs1_g = consts.tile([128, 4, LB, HC, 128], BF16)
        nc.gpsimd.dma_start(cs1_g[:], dr["cs1_g"].ap())
        ones_g = consts.tile([128, 4, 128], BF16)
        nc.gpsimd.dma_start(ones_g[:], dr["ones_g"].ap())
        oh8 = consts.tile([128, 4, 2, 128], FP8)
        nc.gpsimd.dma_start(oh8[:], dr["oh8"].ap())
        ohcols_b = consts.tile([128, 4, 128], BF16)
        nc.vector.tensor_copy(out=ohcols_b[:], in_=oh8[:, :, 0, :])
        ctxb_col = consts.tile([128, LB, HC], F32)
        nc.sync.dma_start(ctxb_col[:], dr["ctxb_col"].ap().rearrange("a p m -> p a m"))
        b1p = consts.tile([128, LB, HC], F32)
        nc.sync.dma_start(b1p[:], dr["b1p"].ap().rearrange("a p m -> p a m"))
        b2col = consts.tile([128, LB, HC], F32)
        nc.sync.dma_start(b2col[:], dr["b2col"].ap().rearrange("a p m -> p a m"))
        padT_s = consts.tile([128, L, HC, 2 * W], BF16)
        nc.gpsimd.dma_start(padT_s[:], dr["padT"].ap().rearrange("l c p w -> p l c w"))
        w1pT_s = consts.tile([128, L, 2, HC, HC, 128], BF16)
        nc.gpsimd.dma_start(
            w1pT_s[:], dr["w1pT"].ap().rearrange("l b c p m n -> p l b c m n"))
        w2T_s = consts.tile([128, L, 2, HC, HC, 128], BF16)
        nc.gpsimd.dma_start(
            w2T_s[:], dr["w2T"].ap().rearrange("l b c p m n -> p l b c m n"))

        # ---- per-batch buffer state ----
        bufs = {}   # b -> dict(x0, xA, xB)
        wp_tiles = {}

        def in_buf(b, l, br):
            d = bufs[b]
            return d["x0"] if l == 0 else (d["xA"][br] if l % 2 == 1 else d["xB"][br])

        def out_buf(b, l, br):
            d = bufs[b]
            return d["xA"][br] if l % 2 == 0 else d["xB"][br]

        # ---- unit emission helpers; unit = (b, l, br) ----
        state = {}  # unit -> dict(h_sb, st_sum, st_sq, negm, rstd)

        def prologue(u):
            b, l, br = u
            if l == 0 and br == 0:
                d = {}
                d["x0"] = xbufs.tile([128, HC, SP], BF16, tag="xB0",
                                     name=f"x0_fm_{b}")
                d["xA"] = [xbufs.tile([128, HC, SP], BF16, tag=f"xA{i}",
                                      name=f"xA{i}_{b}") for i in range(2)]
                d["xB"] = [xbufs.tile([128, HC, SP], BF16, tag=f"xB{i}",
                                      name=f"xB{i}_{b}") for i in range(2)]
                bufs[b] = d
                x0_fm = d["x0"]
                # token-major bf16 staging (gpsimd DMA casts f32->bf16),
                # then PE transposes into the feature-major buffer.
                x0tm = x0tm_p.tile([128, S_ // 128, H], BF16, tag="x0tm",
                                   name=f"x0tm_{b}")
                nc.gpsimd.dma_start(
                    x0tm[:],
                    dr["x0"].ap()[b].rearrange("(g p) h -> p g h", p=128))
                for g2 in range(S_ // 128):
                    pst = ps_misc.tile([128, 2, 128], BF16, tag="misc",
                                       name="pst0")
                    for c in range(HC):
                        nc.tensor.transpose(
                            pst[:, c, :],
                            x0tm[:, g2, c * 128 : (c + 1) * 128], ident[:])
                    col = W + g2 * 128
                    if g2 % 2 == 0:
                        nc.scalar.copy(
                            out=x0_fm[:, :, col : col + 128], in_=pst[:])
                    else:
                        nc.vector.tensor_copy(
                            out=x0_fm[:, :, col : col + 128], in_=pst[:])
            if br == 0:
                wpT = wstream.tile([128, 2, 4, HC, HC, 128], BF16, tag="wpT",
                                   name=f"wpT_{b}_{l}")
                nc.gpsimd.dma_start(
                    wpT[:],
                    dr["wpT"].ap()[l].rearrange("b j c p m n -> p b j c m n"))
                wp_tiles[(b, l)] = wpT
            if not (l == 0 and br == 1):
                buf = in_buf(b, l, br)
                for c in range(HC):
                    nc.gpsimd.tensor_copy(buf[:, c, 0:W], padT_s[:, l, c, 0:W])
                    nc.gpsimd.tensor_copy(
                        buf[:, c, S_ + W : S_ + 2 * W], padT_s[:, l, c, W : 2 * W])

        def emit_A_mm(u, g):
            # ctx matmuls + relu(h) + sq; the LN stat matmuls are emitted
            # separately (emit_A_stats) after B1(prev) to keep them off the
            # PE critical path while ACT/DVE produce h and sq.
            b, l, br = u
            lb = l * 2 + br
            xin = in_buf(b, l, br)
            wpT = wp_tiles[(b, l)]
            off = 0 if br == 0 else W
            st = state.setdefault(u, {})
            if g == 0:
                st["h_sb"] = h_p.tile([128, HC, S_], BF16, tag="h",
                                      name=f"h_{b}_{l}_{br}")
                st["st_sum"] = ps_st.tile([128, 512], F32, tag="st_sum",
                                          name=f"stsum_{b}_{l}_{br}")
                st["st_sq"] = ps_st.tile([128, 512], F32, tag="st_sq",
                                         name=f"stsq_{b}_{l}_{br}")
            h_sb = st["h_sb"]
            t0 = g * 512
            sq8 = sq_p.tile([128, HC, 512], FP8, tag="sq8", name="sq8")
            for m in range(HC):
                psc = pm.tile([128, 512], F32, tag="pm", name="psc")
                for j in range(W + 1):
                    for c in range(HC):
                        nc.tensor.matmul(
                            psc[:], wpT[:, br, j, c, m, :],
                            xin[:, c, t0 + off + j : t0 + off + j + 512],
                            start=(j == 0 and c == 0),
                            stop=(j == W and c == HC - 1))
                nc.scalar.activation(
                    h_sb[:, m, t0 : t0 + 512], psc[:], AF.Relu,
                    bias=ctxb_col[:, lb, m : m + 1])
                nc.vector.tensor_mul(sq8[:, m, :], h_sb[:, m, t0 : t0 + 512],
                                     h_sb[:, m, t0 : t0 + 512])
            st[("sq8", g)] = sq8

        def emit_A_stats(u, g):
            st = state[u]
            h_sb = st["h_sb"]
            sq8 = st.pop(("sq8", g))
            t0 = g * 512
            for m in range(HC):
                nc.tensor.matmul(
                    st["st_sum"][:], ohcols_b[:, g % 4, :],
                    h_sb[:, m, t0 : t0 + 512],
                    start=(g == 0 and m == 0),
                    stop=(g == NG - 1 and m == HC - 1),
                    skip_group_check=True)
            nc.tensor.matmul(
                st["st_sq"][:], oh8[:, g % 4, :, :], sq8[:, :, :],
                start=(g == 0), stop=(g == NG - 1),
                perf_mode=mybir.MatmulPerfMode.DoubleRow,
                skip_group_check=True)

        def emit_R(u):
            st = state[u]
            negm = rows_p.tile([128, 512], BF16, tag="negm", name="negm")
            nc.vector.tensor_scalar_mul(negm[:], st["st_sum"][:], -1.0 / H)
            m2 = rows_p.tile([128, 512], F32, tag="rtmp", name="m2")
            nc.vector.tensor_mul(m2[:], negm[:], negm[:])
            v = rows_p.tile([128, 512], F32, tag="rtmp", name="v")
            nc.vector.scalar_tensor_tensor(
                out=v[:], in0=st["st_sq"][:], scalar=1.0 / H, in1=m2[:],
                op0=ALU.mult, op1=ALU.subtract)
            rstd = rows_p.tile([128, 512], BF16, tag="rstd", name="rstd")
            if arsqrt:
                nc.scalar.activation(
                    rstd[:], v[:], AF.Abs_reciprocal_sqrt, bias=eps_t[:])
            else:
                sig = rows_p.tile([128, 512], F32, tag="rtmp", name="sig")
                nc.scalar.activation(sig[:], v[:], AF.Sqrt, bias=eps_t[:])
                with nc.allow_low_precision(reason="bf16 rstd for matmul rhs"):
                    nc.vector.reciprocal(rstd[:], sig[:])
            st["negm"] = negm
            st["rstd"] = rstd

        def emit_B1(u, g):
            b, l, br = u
            lb = l * 2 + br
            st = state[u]
            h_sb, negm, rstd = st["h_sb"], st["negm"], st["rstd"]
            t0 = g * 512
            gp = 32 * (g % 4)
            psz = [None, None]
            for m in range(HC):
                psz[m] = pm.tile([128, 512], F32, tag="pm", name="psz")
                for c in range(HC):
                    nc.tensor.matmul(
                        psz[m][:], w1pT_s[:, l, br, c, m, :],
                        h_sb[:, c, t0 : t0 + 512], start=(c == 0), stop=False)
                nc.tensor.matmul(
                    psz[m][:], cs1_g[:, g % 4, lb, m, :], negm[:],
                    start=False, stop=True)
            rb = pm.tile([128, 512], F32, tag="pm", name="rb")
            nc.tensor.matmul(
                rb[:], ones_g[:, g % 4, :], rstd[:],
                start=True, stop=True)
            rb_sb = tmp_p.tile([128, 512], F32, tag="rb_sb", name="rb_sb")
            nc.scalar.copy(out=rb_sb[:], in_=rb[:])
            f1_sb = f1_p.tile([128, HC, 512], BF16, tag="f1", name="f1_sb")
            for m in range(HC):
                t_sb = tmp_p.tile([128, 512], F32, tag="t_sb", name="t_sb")
                nc.vector.tensor_mul(t_sb[:], psz[m][:], rb_sb[:])
                nc.scalar.activation(
                    f1_sb[:, m, :], t_sb[:], AF.Relu, bias=b1p[:, lb, m : m + 1])
            st[("f1", g)] = f1_sb

        def emit_B2mm(u, g):
            b, l, br = u
            lb = l * 2 + br
            st = state[u]
            h_sb = st["h_sb"]
            f1_sb = st.pop(("f1", g))
            t0 = g * 512
            if l < L - 1:
                xn_dst = out_buf(b, l, br)
                xn_off = W + t0
            else:
                xn_dst = xn_p.tile([128, HC, 512], BF16, tag="xn", name="xn_st")
                xn_off = 0
            for m in range(HC):
                pso = pm.tile([128, 512], F32, tag="pm", name="pso")
                for c in range(HC):
                    nc.tensor.matmul(
                        pso[:], w2T_s[:, l, br, c, m, :], f1_sb[:, c, :],
                        start=(c == 0), stop=(c == HC - 1))
                nc.vector.scalar_tensor_tensor(
                    out=xn_dst[:, m, xn_off : xn_off + 512], in0=pso[:],
                    scalar=b2col[:, lb, m : m + 1],
                    in1=h_sb[:, m, t0 : t0 + 512], op0=ALU.add, op1=ALU.add)
            st[("xn", g)] = (xn_dst, xn_off)

        def emit_B2store(u, g):
            b, l, br = u
            st = state[u]
            xn_dst, xn_off = st.pop(("xn", g))
            t0 = g * 512
            for s in range(4):
                pst = ps_misc.tile([128, 2, 128], BF16, tag="misc", name="pst")
                for m in range(HC):
                    so = xn_off + s * 128
                    nc.tensor.transpose(
                        pst[:, m, :], xn_dst[:, m, so : so + 128], ident[:])
                tmf = tm_p.tile([128, 2, 128], F32, tag="tmf", name="tmf")
                if s % 2 == 0:
                    nc.scalar.copy(out=tmf[:], in_=pst[:])
                else:
                    nc.vector.tensor_copy(out=tmf[:], in_=pst[:])
                nc.sync.dma_start(
                    dr["out"].ap()[l, b, t0 + s * 128 : t0 + (s + 1) * 128,
                                   br * H : (br + 1) * H],
                    tmf[:])

        # ---- software-pipelined unit stream ----
        # Per 512-token group: Amm(u_i, g) | B1(u_{i-1}, g) | stats(u_i, g) |
        # B2(u_{i-1}, g-1): the PE runs ctx matmuls, then B1 (independent of
        # this group's relu/sq), then the stats (whose ACT/DVE inputs are now
        # ready), then ffn2 — no stall on the relu/sq chain.
        units = [(b, l, br) for b in range(B_local) for l in range(L)
                 for br in range(2)]
        prev = None
        for u in units:
            prologue(u)
            for g in range(NG):
                emit_A_mm(u, g)
                if prev is not None:
                    emit_B1(prev, g)
                if g > 0:
                    emit_A_stats(u, g - 1)
                if prev is not None and g > 0:
                    emit_B2mm(prev, g - 1)
                if prev is not None and g > 1:
                    emit_B2store(prev, g - 2)
            emit_A_stats(u, NG - 1)
            emit_R(u)
            if prev is not None:
                emit_B2mm(prev, NG - 1)
                emit_B2store(prev, NG - 2)
                emit_B2store(prev, NG - 1)
                state.pop(prev)
            prev = u
        for g in range(NG):
            emit_B1(prev, g)
            if g > 0:
                emit_B2mm(prev, g - 1)
            if g > 1:
                emit_B2store(prev, g - 2)
        emit_B2mm(prev, NG - 1)
        emit_B2store(prev, NG - 2)
        emit_B2store(prev, NG - 1)


# ---- SPMD wrapper ----
from concourse.bass_utils import run_bass_kernel_spmd

B, S, L_ = 32, 2048, 3
N_CORES = 8
B_LOCAL = B // N_CORES
_NC_CACHE = {}


def _get_nc():
    key = (B_LOCAL, S)
    if key not in _NC_CACHE:
        _NC_CACHE[key] = build_nc(B_LOCAL, S, L_)
    return _NC_CACHE[key]


def run(inputs, **spmd_kwargs):
    prep = prep_weights(inputs, L_)
    x = np.ascontiguousarray(np.asarray(inputs["inputs"], np.float32))
    nc = _get_nc()
    in_maps = []
    for core in range(N_CORES):
        m = {"x0": x[core * B_LOCAL : (core + 1) * B_LOCAL]}
        m.update(prep)
        in_maps.append(m)
    res = run_bass_kernel_spmd(nc, in_maps, list(range(N_CORES)), **spmd_kwargs)
    out = np.concatenate([res.results[i]["out"] for i in range(N_CORES)], axis=1)
    return out, res


def kernel(**inputs):
    out, _ = run(inputs)
    return out
